# revision 4
# baseline (speedup 1.0000x reference)
"""Trainium2 Bass kernel for 2-layer grouped LSTM — v3: hardware-loop version.

Same dataflow as v2 (fused 2-layer recurrence, chain-sets at partition
bases {0,32,64,96}, set-major hT ring, 16-step gih block GEMMs) but the
1018-iteration recurrence is compressed into a 32-iteration body inside a
tc.For_i hardware loop (30 trips), with an unrolled 32-iter prologue and
26-iter epilogue.  Layer-1's input-gate GEMM runs as an unrolled phase-A
pass into DRAM (stationary operands need static offsets); per-step slices
stream back via dynamic-offset DMAs.  ~10x fewer instructions than full
unrolling, which dominates per-call program load time in this harness.
"""

import sys
from contextlib import ExitStack

import numpy as np

sys.path.insert(0, "/opt/trn_rl_repo")

B_FULL, CHANNEL, T, FEATURE = 64, 64, 1000, 5
G, H = 2, 160
G4 = 4 * H  # 640
D = CHANNEL * FEATURE  # 320
NCORES = 8
B = B_FULL // NCORES  # 8
BT = B * T
SPAN = 104
PB = [0, 32, 64, 96]
NR = 16              # hT ring slots (= gih block size)
LAG = 18             # layer-2 trails layer-1 by this many iterations
OB = 8               # output DMA batch (steps)
BODY = 32            # iterations per hardware-loop body
PF = 2               # gih1 DMA prefetch distance (iterations)

_CACHE = {}


def _perm_ifog():
    return np.concatenate([
        np.arange(0, 160), np.arange(160, 320),
        np.arange(480, 640), np.arange(320, 480),
    ])


def _build():
    import concourse.tile as tile
    from concourse import bacc, mybir
    from concourse.bass import ds
    from concourse.masks import make_identity

    f32 = mybir.dt.float32
    bf16 = mybir.dt.bfloat16
    SIG = mybir.ActivationFunctionType.Sigmoid
    TANH = mybir.ActivationFunctionType.Tanh
    COPY = mybir.ActivationFunctionType.Copy

    nc = bacc.Bacc(None, target_bir_lowering=False)

    # consolidated inputs: one xT pack and one weight pack per core
    xTd = nc.dram_tensor("xT", [322, BT], bf16, kind="ExternalInput")
    Wd = nc.dram_tensor("W", [1284, G4], bf16, kind="ExternalInput")
    out = nc.dram_tensor("out", [BT, D], bf16, kind="ExternalOutput")

    with tile.TileContext(nc) as tc, ExitStack() as top:
        const = top.enter_context(tc.tile_pool(name="const", bufs=1))
        dram = top.enter_context(tc.tile_pool(name="dram", bufs=1,
                                              space="DRAM"))

        id_f = const.tile([128, 128], f32)
        make_identity(nc, id_f[:])
        idn = const.tile([128, 128], bf16)
        nc.vector.tensor_copy(idn[:], id_f[:])

        def sb2(dram_t, r0, p, nm):
            t_ = const.tile([p, BT], bf16, name=nm, tag=nm)
            nc.sync.dma_start(t_[:], dram_t[r0:r0 + p, 0:BT])
            return t_

        def sbw(r0, p, nm):
            t_ = const.tile([p, G4], bf16, name=nm, tag=nm)
            nc.sync.dma_start(t_[:], Wd[r0:r0 + p, 0:G4])
            return t_

        xa = [sb2(xTd, 161 * g, 128, f"xa{g}") for g in range(G)]
        xb = [sb2(xTd, 161 * g + 128, 33, f"xb{g}") for g in range(G)]
        wA = [sbw(642 * g + 0, 128, f"wA{g}") for g in range(G)]
        wB = [sbw(642 * g + 128, 33, f"wB{g}") for g in range(G)]
        h1A = [sbw(642 * g + 161, 80, f"h1A{g}") for g in range(G)]
        h1B = [sbw(642 * g + 241, 80, f"h1B{g}") for g in range(G)]
        w2p1 = [sbw(642 * g + 321, 80, f"w2p1{g}") for g in range(G)]
        w2p2 = [sbw(642 * g + 401, 80, f"w2p2{g}") for g in range(G)]
        h2A = [sbw(642 * g + 481, 80, f"h2A{g}") for g in range(G)]
        h2B = [sbw(642 * g + 561, 81, f"h2B{g}") for g in range(G)]

        # hT ring, set-major: col = set*128 + slot*8 + batch
        hTA = const.tile([80, 512], bf16)
        hTB = const.tile([81, 512], bf16)
        nc.gpsimd.memset(hTA[:], 0.0)
        nc.gpsimd.memset(hTB[:], 1.0)
        nc.gpsimd.memset(hTB[0:80, :], 0.0)

        cT = [const.tile([SPAN, H], f32, name=f"c{i}") for i in range(2)]
        nc.gpsimd.memset(cT[0][:], 0.0)
        nc.gpsimd.memset(cT[1][:], 0.0)

        g2blk = [[const.tile([128, G4], bf16, name=f"g2b{g}_{i}")
                  for i in range(2)] for g in range(G)]
        hob = [const.tile([SPAN, H * OB], bf16, name=f"hob{i}")
               for i in range(2)]
        # layer-1 gih 16-step chunk tiles (ping-pong)
        g1c = [[const.tile([128, G4], bf16, name=f"g1c{g}_{i}")
                for i in range(2)] for g in range(G)]
        zt = const.tile([64, G4], bf16)
        nc.gpsimd.memset(zt[:], 0.0)

        gih1d = [dram.tile([BT + 64, G4], bf16, name=f"gih1d{g}",
                           tag=f"gih1d{g}") for g in range(G)]

        with tc.tile_pool(name="po", bufs=2, space="PSUM") as pso, \
             tc.tile_pool(name="pg", bufs=2, space="PSUM") as psg, \
             tc.tile_pool(name="pt", bufs=2, space="PSUM") as pstp, \
             tc.tile_pool(name="pbo", bufs=1, space="PSUM") as psbo, \
             tc.tile_pool(name="pbg", bufs=1, space="PSUM") as psbg, \
             tc.tile_pool(name="wk", bufs=3) as wp, \
             tc.tile_pool(name="wk2", bufs=2) as wp2:

            # ---- phase A: gih1 = xT^T @ Wih1 + b1 -> DRAM ---------------
            for m0 in range(0, BT, 128):
                mc = min(128, BT - m0)
                for g in range(G):
                    bo = psbo.tile([128, 480], f32, tag="pbo")
                    bg = psbg.tile([128, 160], f32, tag="pbg")
                    for (n0, n1, pdst) in ((0, 480, bo), (480, 640, bg)):
                        nc.tensor.matmul(pdst[0:mc, :],
                                         xa[g][:, m0:m0 + mc],
                                         wA[g][:, n0:n1],
                                         start=True, stop=False)
                        nc.tensor.matmul(pdst[0:mc, :],
                                         xb[g][:, m0:m0 + mc],
                                         wB[g][:, n0:n1],
                                         start=False, stop=True)
                    stg = wp2.tile([128, G4], bf16, tag="stg")
                    nc.vector.tensor_copy(stg[0:mc, 0:480], bo[0:mc, :])
                    nc.scalar.activation(stg[0:mc, 480:640], bg[0:mc, :],
                                         COPY)
                    nc.sync.dma_start(gih1d[g][m0:m0 + mc, :], stg[0:mc, :])

            # zero padded tail rows; preload chunk 0
            for g in range(G):
                nc.sync.dma_start(gih1d[g][BT:BT + 64, :], zt[:])
                nc.sync.dma_start(g1c[g][0][:], gih1d[g][0:128, :])

            def xgemm2(bidx, bm):
                """gih2 block GEMM for layer-2 block bidx (bm = M rows)."""
                for g2 in range(G):
                    src = hTA if g2 == 0 else hTB
                    bo = psbo.tile([128, 480], f32, tag="pbo")
                    bg = psbg.tile([128, 160], f32, tag="pbg")
                    for (n0, n1, pdst) in ((0, 480, bo), (480, 640, bg)):
                        nc.tensor.matmul(pdst[0:bm, :], src[0:80, 0:bm],
                                         w2p1[g2][:, n0:n1],
                                         start=True, stop=False)
                        nc.tensor.matmul(pdst[0:bm, :],
                                         src[0:80, 128:128 + bm],
                                         w2p2[g2][:, n0:n1],
                                         start=False, stop=True)
                    dst = g2blk[g2][bidx % 2]
                    nc.vector.tensor_copy(dst[0:bm, 0:480], bo[0:bm, :])
                    nc.scalar.activation(dst[0:bm, 480:640], bg[0:bm, :],
                                         COPY)

            def emit_iter(t, j=None, blkrv=None):
                """One recurrence iteration.  t: static iteration index
                (prologue/epilogue) or base-relative when j/blkrv given:
                then t = BODY*blk + j at runtime and `t` passed here must
                be a representative with the same residues (use t=j+BODY).
                blkrv: ScalarValue equal to blk*256 (row offset scale)."""
                dyn = blkrv is not None
                do_l1 = True if dyn else t < T
                do_l2 = True if dyn else t >= LAG
                slot, prev = t % NR, (t - 1) % NR
                lo, hi = ((0, 40) if t < LAG else
                          ((64, SPAN) if t >= T else (0, SPAN)))

                # L2 gih block GEMM every NR iters (consumed LAG-NR=2 later)
                if t % NR == 0 and (dyn or NR <= t <= T + NR):
                    bidx = (t - NR) // NR
                    if dyn:
                        # only the g2blk parity matters; it depends on j only
                        xgemm2(bidx, 128)
                    elif NR * bidx < T:
                        xgemm2(bidx, min(NR, T - NR * bidx) * B)

                # L1 gih prefetch DMA for step t+PF
                if t % NR == 0:
                    cn = t // NR + 1  # next gih1 chunk
                    if dyn:
                        # runtime rows = rv + (cn*128 - BODY*B)
                        for g in range(G):
                            nc.gpsimd.dma_start(
                                g1c[g][cn % 2][:],
                                gih1d[g][ds(blkrv + cn * 128 - BODY * B,
                                            128), :])
                    elif NR * cn < T:
                        mc2 = min(128, BT - cn * 128)
                        for g in range(G):
                            nc.sync.dma_start(
                                g1c[g][cn % 2][0:mc2, :],
                                gih1d[g][cn * 128:cn * 128 + mc2, :])

                po = pso.tile([SPAN, 480], f32, tag="po")
                pg = psg.tile([SPAN, 160], f32, tag="pg")

                sets = []
                if do_l1:
                    k1 = (t % NR) * B
                    c1 = (t // NR) % 2
                    for g in range(G):
                        pc = g * 128 + prev * 8
                        sets.append((PB[g], [
                            (idn[0:128, k1:k1 + 8], g1c[g][c1]),
                            (hTA[0:80, pc:pc + 8], h1A[g]),
                            (hTB[0:80, pc:pc + 8], h1B[g]),
                        ]))
                if do_l2:
                    tau = t - LAG
                    k8 = (tau % NR) * B
                    bi = (tau // NR) % 2
                    for g2 in range(G):
                        s = 2 + g2
                        pc = s * 128 + prev * 8
                        sets.append((PB[s], [
                            (idn[0:128, k8:k8 + 8], g2blk[g2][bi]),
                            (hTA[0:80, pc:pc + 8], h2A[g2]),
                            (hTB[0:81, pc:pc + 8], h2B[g2]),
                        ]))

                for (n0, n1, pdst) in ((0, 480, po), (480, 640, pg)):
                    for (base, kparts) in sets:
                        for ki, (lh, mv) in enumerate(kparts):
                            nc.tensor.matmul(
                                pdst[base:base + 8, :], lh, mv[:, n0:n1],
                                start=(ki == 0), stop=(ki == len(kparts) - 1),
                                tile_position=(0, base))

                sg = wp.tile([SPAN, 480], bf16, tag="sg")
                nc.scalar.activation(sg[lo:hi, :], po[lo:hi, :], SIG)
                tg = wp.tile([SPAN, 160], bf16, tag="tg")
                nc.scalar.activation(tg[lo:hi, :], pg[lo:hi, :], TANH)

                c_in, c_out = cT[t % 2], cT[(t + 1) % 2]
                t2 = wp.tile([SPAN, H], f32, tag="t2")
                nc.vector.tensor_mul(t2[lo:hi, :], sg[lo:hi, 160:320],
                                     c_in[lo:hi, :])
                t1 = wp.tile([SPAN, H], bf16, tag="t1")
                nc.vector.tensor_mul(t1[lo:hi, :], sg[lo:hi, 0:160],
                                     tg[lo:hi, :])
                nc.vector.tensor_add(c_out[lo:hi, :], t1[lo:hi, :],
                                     t2[lo:hi, :])
                th = wp.tile([SPAN, H], bf16, tag="th")
                nc.scalar.activation(th[lo:hi, :], c_out[lo:hi, :], TANH)
                ho = hob[(t // OB) % 2]
                oc = (t % OB) * H
                nc.vector.tensor_mul(ho[lo:hi, oc:oc + H],
                                     sg[lo:hi, 320:480], th[lo:hi, :])

                if do_l2 and (t % OB == OB - 1 or t == T + LAG - 1):
                    nb = (t % OB) + 1
                    n2 = nb if dyn else min(nb, t - LAG + 1)
                    tau1 = t - LAG
                    tau0 = tau1 - n2 + 1
                    oc0 = ((t - n2 + 1) % OB) * H
                    srcg0 = ho[64:72, oc0:oc0 + n2 * H].rearrange(
                        "b (s d) -> b s d", d=H)
                    srcg1 = ho[96:SPAN, oc0:oc0 + n2 * H].rearrange(
                        "b (s d) -> b s d", d=H)
                    if dyn:
                        dsl = ds(blkrv + (tau0 - BODY) * B, n2 * B)
                        d3 = out[dsl, :].rearrange("(s b) d -> b s d", b=B)
                    else:
                        d3 = out[tau0 * B:(tau1 + 1) * B, :].rearrange(
                            "(s b) d -> b s d", b=B)
                    nc.gpsimd.dma_start(d3[:, :, 0:160], srcg0)
                    nc.gpsimd.dma_start(d3[:, :, 160:320], srcg1)

                if dyn or t < T + LAG - 1:
                    n = hi - lo
                    s0, s1 = lo // 32, (hi + 31) // 32
                    ptn = pstp.tile([80, 256], bf16, tag="pt")
                    nc.tensor.transpose(ptn[0:80, 0:n],
                                        ho[lo:hi, oc:oc + 80],
                                        idn[lo:hi, lo:hi])
                    nc.tensor.transpose(ptn[0:80, 128:128 + n],
                                        ho[lo:hi, oc + 80:oc + 160],
                                        idn[lo:hi, lo:hi])
                    srcA = ptn[0:80, 0:128].rearrange(
                        "p (s c) -> p s c", c=32)[:, 0:s1 - s0, 0:8]
                    srcB = ptn[0:80, 128:256].rearrange(
                        "p (s c) -> p s c", c=32)[:, 0:s1 - s0, 0:8]
                    dstA = hTA[0:80, :].rearrange(
                        "p (s c) -> p s c", c=128)[:, s0:s1,
                                                   slot * 8:slot * 8 + 8]
                    dstB = hTB[0:80, :].rearrange(
                        "p (s c) -> p s c", c=128)[:, s0:s1,
                                                   slot * 8:slot * 8 + 8]
                    nc.vector.tensor_copy(dstA, srcA)
                    nc.scalar.activation(dstB, srcB, COPY)

            # ---- prologue: iterations 0..BODY-1 -------------------------
            for t in range(BODY):
                emit_iter(t)

            # ---- main loop: blk = 1..T//BODY-1, t = BODY*blk + j --------
            with tc.For_i(BODY * B, (T // BODY) * BODY * B, BODY * B) as rv:
                # rv == blk * BODY * B == row offset of the body's start
                for j in range(BODY):
                    emit_iter(BODY + j, j=j, blkrv=rv)

            # ---- epilogue: iterations T..T+LAG-1 (plus last loop block
            # already covered t up to (T//BODY)*BODY - 1 = 991) ------------
            for t in range((T // BODY) * BODY, T + LAG):
                emit_iter(t)

    nc.finalize()
    return nc


def _get_nc():
    if "nc" not in _CACHE:
        _CACHE["nc"] = _build()
    return _CACHE["nc"]


def _prep_weights(Wih1, Whh1, b1, Wih2, Whh2, b2):
    import ml_dtypes

    bf = ml_dtypes.bfloat16
    perm = _perm_ifog()
    W = np.empty((1284, G4), bf)
    dd = np.arange(80)
    p1_rows = (dd // 5) * 10 + dd % 5
    for g in range(G):
        r = 642 * g
        wp = np.concatenate([np.asarray(Wih1[g], np.float32)[:, perm],
                             np.asarray(b1[g], np.float32)[perm][None]], 0)
        W[r:r + 161] = wp.astype(bf)
        w = np.asarray(Whh1[g], np.float32)[:, perm]
        W[r + 161:r + 321] = w.astype(bf)
        w2p = np.asarray(Wih2[g], np.float32)[:, perm]
        W[r + 321:r + 401] = w2p[p1_rows].astype(bf)
        W[r + 401:r + 481] = w2p[p1_rows + 5].astype(bf)
        w = np.asarray(Whh2[g], np.float32)[:, perm]
        W[r + 481:r + 561] = w[0:80].astype(bf)
        W[r + 561:r + 641] = w[80:160].astype(bf)
        W[r + 641] = np.asarray(b2[g], np.float32)[perm].astype(bf)
    return {"W": W}


def _kernel_impl(input, Wih1, Whh1, b1, Wih2, Whh2, b2, snd_index,
                 trace=False):
    import ml_dtypes

    from concourse.bass_utils import run_bass_kernel_spmd

    bf = ml_dtypes.bfloat16
    input = np.asarray(input)
    wmap = _prep_weights(Wih1, Whh1, b1, Wih2, Whh2, b2)

    x = np.ascontiguousarray(
        np.transpose(input, (0, 2, 1, 3))).reshape(B_FULL, T, D)

    in_maps = []
    for c in range(NCORES):
        xc = x[c * B:(c + 1) * B]
        xtm = np.ascontiguousarray(xc.transpose(1, 0, 2)).reshape(BT, D)
        xT = np.empty((322, BT), bf)
        for g in range(G):
            xT[161 * g:161 * g + 160] = xtm[:, g * H:(g + 1) * H].T
            xT[161 * g + 160] = 1.0
        in_maps.append({"W": wmap["W"], "xT": xT})

    nc = _get_nc()
    res = run_bass_kernel_spmd(nc, in_maps, core_ids=list(range(NCORES)),
                               trace=trace)
    outs = []
    for c in range(NCORES):
        o = np.asarray(res.results[c]["out"]).astype(np.float32)
        o = o.reshape(T, B, D).transpose(1, 0, 2)
        outs.append(o)
    full = np.concatenate(outs, 0)
    full = full.reshape(B_FULL, T, CHANNEL, FEATURE).transpose(0, 2, 1, 3)
    return np.ascontiguousarray(full), res


def kernel(input, Wih1, Whh1, b1, Wih2, Whh2, b2, snd_index):
    out, _ = _kernel_impl(input, Wih1, Whh1, b1, Wih2, Whh2, b2, snd_index)
    return out


def kernel_traced(input, Wih1, Whh1, b1, Wih2, Whh2, b2, snd_index):
    return _kernel_impl(input, Wih1, Whh1, b1, Wih2, Whh2, b2, snd_index,
                        trace=True)
